# revision 4
# baseline (speedup 1.0000x reference)
"""Trainium2 Bass kernel for nn_DataEmbedding_Stats.

Computation: rolling-window stats (window=24, replicate-padded) over
x (B,S,7) -> 35 features -> circular conv1d(k=3) -> (B,S,512).

Strategy (8 NeuronCores, pure data parallel over batch, 4 batches/core):
 - x loaded into [112, 1047] SBUF layout: partition = j*28 + b*7 + c
   (j = 1024-seq chunk, b = local batch, c = channel), free = seq + 23 halo.
 - rolling sum/sumsq/max/min via log-doubling shifted ops on DVE
   (window 24 = combine(16-window, 8-window shifted by 16)).
 - std = sqrt(max(SQ24 - S24^2/24, 0)/23); mean folded into conv weights
   as S24 * (W_mean/24) on host.
 - conv as matmul: per 128 positions, out[128,512] = F3[:,t+1:t+129].T @ Wt
   where F3 [106,4098] holds the 3 conv-tap-shifted copies of the 35
   features (circular wrap) plus a ones-row (bias folded as contraction
   row 105). float32r operands -> full PE speed at N=512.
 - PSUM -> SBUF copies split between DVE/ACT, 1MB output DMAs.
"""

import numpy as np

try:
    import concourse.bass as bass  # noqa: F401
except ImportError:
    import sys

    for _p in ("/opt/trn_rl_repo", "/root/.axon_site/_ro/trn_rl_repo"):
        if _p not in sys.path:
            sys.path.insert(0, _p)

B, S, C, W, D = 32, 4096, 7, 24, 512
NCORES = 8
BSH = B // NCORES          # batches per core
NJ = 4                     # seq chunks per batch row-group
CH = S // NJ               # 1024
HALO = W - 1               # 23
XCOLS = CH + HALO          # 1047
NF = 5 * C                 # 35 features
K = 3 * NF + 1             # 106 contraction rows (ones row last)
F3W = S + 2                # 4098
NT = S // 128              # 32 position tiles per batch
DVE_COLS = 224             # psum-copy split: DVE takes [0:224], ACT [224:512]

_CACHE = {}


def _build():
    import concourse.bacc as bacc
    import concourse.tile as tile
    from concourse import mybir

    f32 = mybir.dt.float32
    f32r = mybir.dt.float32r
    Alu = mybir.AluOpType
    Act = mybir.ActivationFunctionType

    nc = bacc.Bacc(
        "TRN2",
        target_bir_lowering=False,
        debug=False,
        enable_asserts=False,
        num_devices=NCORES,
    )

    x_d = nc.dram_tensor("x", (BSH, S, C), f32, kind="ExternalInput")
    wt_d = nc.dram_tensor("wt", (K, D), f32r, kind="ExternalInput")
    ones_d = nc.dram_tensor("ones", (1, F3W), f32r, kind="ExternalInput")
    y_d = nc.dram_tensor("y", (BSH, S, D), f32, kind="ExternalOutput")

    with tile.TileContext(nc) as tc:
        with (
            tc.tile_pool(name="stats", bufs=1) as pst,
            tc.tile_pool(name="f3p", bufs=1) as pf3,
            tc.tile_pool(name="wtp", bufs=1) as pwt,
            tc.tile_pool(name="psum", bufs=8, space="PSUM") as pps,
            tc.tile_pool(name="outp", bufs=3) as pout,
        ):
            wt = pwt.tile([K, D], f32r, tag="wt")
            nc.sync.dma_start(wt[:], wt_d.ap())

            X = pst.tile([112, XCOLS], f32, tag="X")
            T1 = pst.tile([112, XCOLS], f32, tag="T1")
            T2 = pst.tile([112, XCOLS], f32, tag="T2")
            T3 = pst.tile([112, XCOLS], f32, tag="T3")
            T4 = pst.tile([112, XCOLS], f32, tag="T4")
            S24 = pst.tile([112, XCOLS], f32, tag="S24")
            MX = pst.tile([112, XCOLS], f32, tag="MX")
            MN = pst.tile([112, XCOLS], f32, tag="MN")
            SD = pst.tile([112, XCOLS], f32, tag="SD")

            F3 = [pf3.tile([K, F3W], f32r, tag=f"F3_{b}", name=f"F3_{b}") for b in range(BSH)]
            for b in range(BSH):
                nc.sync.dma_start(F3[b][K - 1 : K, :], ones_d.ap())

            # ---- load x: gather-transpose DMAs (seq-major HBM -> [c, s])
            for b in range(BSH):
                for j in range(NJ):
                    p0 = 28 * j + 7 * b
                    nc.sync.dma_start(
                        X[p0 : p0 + 7, HALO:XCOLS],
                        x_d.ap()[b, CH * j : CH * (j + 1), :].rearrange("s c -> c s"),
                    )
                    if j > 0:
                        nc.sync.dma_start(
                            X[p0 : p0 + 7, 0:HALO],
                            x_d.ap()[b, CH * j - HALO : CH * j, :].rearrange(
                                "s c -> c s"
                            ),
                        )
            # j=0 halo: replicate x[b,0,c] into cols 0..22 (j=0 rows for all
            # batches are partitions 0..27 -> single op, base partition 0)
            nc.vector.tensor_scalar(
                X[0 : 7 * BSH, 0:HALO],
                X[0 : 7 * BSH, HALO : 2 * HALO],
                0.0,
                X[0 : 7 * BSH, HALO : HALO + 1],
                Alu.mult,
                Alu.add,
            )

            # ---- rolling stats (all [112, *], shifts along free dim)
            E = XCOLS  # 1047

            def tt(dst, d0, a, a0, bsrc, b0, n, op):
                nc.vector.tensor_tensor(
                    dst[:, d0 : d0 + n], a[:, a0 : a0 + n], bsrc[:, b0 : b0 + n], op
                )

            # sum chain: A(T1) B(T2) C(T3) D(T1) S24
            tt(T1, 1, X, 1, X, 0, E - 1, Alu.add)
            tt(T2, 3, T1, 3, T1, 1, E - 3, Alu.add)
            tt(T3, 7, T2, 7, T2, 3, E - 7, Alu.add)
            tt(T1, 15, T3, 15, T3, 7, E - 15, Alu.add)
            tt(S24, 23, T1, 23, T3, 7, E - 23, Alu.add)
            # squares: SQX(T4 via ACT), A2(T1) B2(T2) C2(T3) D2(T1) SQ24(T2)
            nc.scalar.square(T4[:, 0:E], X[:, 0:E])
            tt(T1, 1, T4, 1, T4, 0, E - 1, Alu.add)
            tt(T2, 3, T1, 3, T1, 1, E - 3, Alu.add)
            tt(T3, 7, T2, 7, T2, 3, E - 7, Alu.add)
            tt(T1, 15, T3, 15, T3, 7, E - 15, Alu.add)
            tt(T2, 23, T1, 23, T3, 7, E - 23, Alu.add)
            # mean^2*24 = (S24/sqrt(24))^2 -> T4 ; var_raw(T1) ; clamp(T2) ; SD
            nc.scalar.activation(
                T4[:, 23:E], S24[:, 23:E], Act.Square, 0.0, float(W**-0.5)
            )
            tt(T1, 23, T2, 23, T4, 23, E - 23, Alu.subtract)
            nc.vector.tensor_scalar(
                T2[:, 23:E], T1[:, 23:E], 0.0, None, Alu.max
            )
            nc.scalar.activation(
                SD[:, 23:E], T2[:, 23:E], Act.Sqrt, 0.0, 1.0 / (W - 1)
            )
            # max chain: MA(T1) MB(T3) MC(T1) MD(T3) MX
            tt(T1, 1, X, 1, X, 0, E - 1, Alu.max)
            tt(T3, 3, T1, 3, T1, 1, E - 3, Alu.max)
            tt(T1, 7, T3, 7, T3, 3, E - 7, Alu.max)
            tt(T3, 15, T1, 15, T1, 7, E - 15, Alu.max)
            tt(MX, 23, T3, 23, T1, 7, E - 23, Alu.max)
            # min chain: NA(T4) NB(T2) NC(T4) ND(T2) MN
            tt(T4, 1, X, 1, X, 0, E - 1, Alu.min)
            tt(T2, 3, T4, 3, T4, 1, E - 3, Alu.min)
            tt(T4, 7, T2, 7, T2, 3, E - 7, Alu.min)
            tt(T2, 15, T4, 15, T4, 7, E - 15, Alu.min)
            tt(MN, 23, T2, 23, T4, 7, E - 23, Alu.min)

            # ---- build F3 operands (per batch)
            stats = [X, S24, MX, MN, SD]
            for b in range(BSH):
                for t, st in enumerate(stats):
                    for k in range(3):
                        r0 = 35 * k + 7 * t
                        for j in range(NJ):
                            nc.sync.dma_start(
                                F3[b][
                                    r0 : r0 + 7,
                                    (2 - k) + CH * j : (2 - k) + CH * (j + 1),
                                ],
                                st[
                                    28 * j + 7 * b : 28 * j + 7 * b + 7, HALO : HALO + CH
                                ].bitcast(f32r),
                            )
                        # circular wraps
                        if k == 0:  # col 1 <- seq 4095
                            nc.sync.dma_start(
                                F3[b][r0 : r0 + 7, 1:2],
                                st[
                                    28 * 3 + 7 * b : 28 * 3 + 7 * b + 7,
                                    XCOLS - 1 : XCOLS,
                                ].bitcast(f32r),
                            )
                        elif k == 2:  # col 4096 <- seq 0
                            nc.sync.dma_start(
                                F3[b][r0 : r0 + 7, S : S + 1],
                                st[7 * b : 7 * b + 7, HALO : HALO + 1].bitcast(f32r),
                            )

            # ---- conv matmuls + copies + stores
            for b in range(BSH):
                for g in range(NT // 4):
                    stage = pout.tile([128, 4 * D], f32, tag="stage")
                    for q in range(4):
                        t0 = 128 * (4 * g + q)
                        ps = pps.tile([128, D], f32, tag="ps")
                        nc.tensor.matmul(
                            ps[:],
                            F3[b][:, t0 + 1 : t0 + 129],
                            wt[:],
                            start=True,
                            stop=True,
                        )
                        c0 = D * q
                        nc.vector.tensor_copy(
                            stage[:, c0 : c0 + DVE_COLS], ps[:, 0:DVE_COLS]
                        )
                        nc.scalar.copy(
                            stage[:, c0 + DVE_COLS : c0 + D], ps[:, DVE_COLS:D]
                        )
                    nc.sync.dma_start(
                        y_d.ap()[b, 512 * g : 512 * (g + 1), :].rearrange(
                            "(q p) d -> p q d", p=128
                        ),
                        stage[:].rearrange("p (q d) -> p q d", q=4),
                    )

    nc.compile()
    return nc


def _prep_host(W_conv, b_conv):
    wt = np.empty((K, D), np.float32)
    wkf = np.ascontiguousarray(W_conv.transpose(2, 1, 0)).copy()  # (3, 35, 512)
    wkf[:, C : 2 * C, :] *= 1.0 / W  # fold mean = S24/24 into weights
    wt[: K - 1] = wkf.reshape(3 * NF, D)
    wt[K - 1] = b_conv.astype(np.float32)
    return wt


def _run(x, W_conv, b_conv, trace=False, **kw):
    from concourse import bass_utils

    if "nc" not in _CACHE:
        _CACHE["nc"] = _build()
    nc = _CACHE["nc"]

    wt = _prep_host(np.asarray(W_conv), np.asarray(b_conv))
    ones = np.ones((1, F3W), np.float32)
    x = np.ascontiguousarray(np.asarray(x, np.float32))
    in_maps = [
        {"x": x[BSH * i : BSH * (i + 1)], "wt": wt, "ones": ones}
        for i in range(NCORES)
    ]
    res = bass_utils.run_bass_kernel_spmd(
        nc, in_maps, core_ids=list(range(NCORES)), trace=trace, **kw
    )
    out = np.concatenate([r["y"] for r in res.results], axis=0)
    return out, res


def kernel(x, x_mark=None, W_conv=None, b_conv=None, **_unused):
    out, _ = _run(x, W_conv, b_conv, trace=False)
    return out


# revision 8
# speedup vs baseline: 1.5299x; 1.5299x over previous
"""Trainium2 Bass kernel for nn_DataEmbedding_Stats.

Computation: rolling-window stats (window=24, replicate-padded) over
x (B,S,7) -> 35 features -> circular conv1d(k=3) -> (B,S,512).

Strategy (8 NeuronCores, pure data parallel over batch, 4 batches/core):
 - x loaded contiguously (28B runs) into [128,128] staging tiles, then
   PE-transposed so channels land on partitions: X [128, 1047] layout,
   partition = 32j + 7b + c (j = 1024-seq chunk, b = local batch,
   c = channel), free = seq within chunk + 23-halo.
 - rolling sum/sumsq/max/min via log-doubling shifted ops on DVE
   (window 24 = combine(16-window, 8-window shifted by 16)).
 - std = sqrt(max(SQ24 - S24^2/24, 0)/23); mean folded into conv weights
   as S24 * (W_mean/24) on host.
 - per-stat contiguous ST2 [28, 4100] tiles (partition = 7b+c, col m =
   feats at seq (m-2) mod 4096) built with GPSIMD copies; F3 [106, 4098]
   per batch = 3 conv-tap shifts of the 35 features (circular) + ones
   row (bias folded as contraction row 105) via 15 wide DMAs per batch.
 - conv as matmul: per 128 positions, out[128,512] = F3[:,t+1:t+129].T
   @ Wt, float32r operands -> full PE speed at N=512.
 - PSUM -> SBUF copies split between DVE/ACT, 2MB output DMAs.
"""

import numpy as np

try:
    import concourse.bass as bass  # noqa: F401
except ImportError:
    import sys

    for _p in ("/opt/trn_rl_repo", "/root/.axon_site/_ro/trn_rl_repo"):
        if _p not in sys.path:
            sys.path.insert(0, _p)

B, S, C, W, D = 32, 4096, 7, 24, 512
NCORES = 8
BSH = B // NCORES          # batches per core
NJ = 4                     # seq chunks (row groups of 32 partitions)
CH = S // NJ               # 1024
HALO = W - 1               # 23
XCOLS = CH + HALO          # 1047
NF = 5 * C                 # 35 features
K = 3 * NF + 1             # 106 contraction rows (ones row last)
F3W = S + 2                # 4098
ST2W = S + 4               # 4100: col m = feats[(m-2) mod S]
NT = S // 128              # 32 position tiles per batch
NTR = S // 512             # 8 PE transposes (each covers 512 seq x 4 batch)
DVE_COLS = 224             # psum-copy split: DVE [0:224], ACT [224:512]
OUTG = 8                   # position tiles per output staging tile

_CACHE = {}


def _build():
    import concourse.bacc as bacc
    import concourse.tile as tile
    from concourse import mybir

    f32 = mybir.dt.float32
    f32r = mybir.dt.float32r
    Alu = mybir.AluOpType
    Act = mybir.ActivationFunctionType

    nc = bacc.Bacc(
        "TRN2",
        target_bir_lowering=False,
        debug=False,
        enable_asserts=False,
        num_devices=NCORES,
    )

    x_d = nc.dram_tensor("x", (BSH, S, C), f32, kind="ExternalInput")
    wt_d = nc.dram_tensor("wt", (K, D), f32r, kind="ExternalInput")
    ones_d = nc.dram_tensor("ones", (1, F3W), f32r, kind="ExternalInput")
    id_d = nc.dram_tensor("ident", (128, 128), f32, kind="ExternalInput")
    y_d = nc.dram_tensor("y", (BSH, S, D), f32, kind="ExternalOutput")

    with tile.TileContext(nc) as tc:
        with (
            tc.tile_pool(name="stats", bufs=1) as pst,
            tc.tile_pool(name="st2p", bufs=1) as pst2,
            tc.tile_pool(name="f3p", bufs=2) as pf3,
            tc.tile_pool(name="wtp", bufs=1) as pwt,
            tc.tile_pool(name="stage_in", bufs=3) as pstg,
            tc.tile_pool(name="psT", bufs=2, space="PSUM") as psT,
            tc.tile_pool(name="psum", bufs=6, space="PSUM") as pps,
            tc.tile_pool(name="outp", bufs=2) as pout,
        ):
            wt = pwt.tile([K, D], f32r, tag="wt")
            nc.sync.dma_start(wt[:], wt_d.ap())
            ident = pwt.tile([128, 128], f32, tag="ident")
            nc.sync.dma_start(ident[:], id_d.ap())

            X = pst.tile([128, XCOLS], f32, tag="X")
            T1 = pst.tile([128, XCOLS], f32, tag="T1")
            T2 = pst.tile([128, XCOLS], f32, tag="T2")
            T3 = pst.tile([128, XCOLS], f32, tag="T3")
            T4 = pst.tile([128, XCOLS], f32, tag="T4")
            S24 = pst.tile([128, XCOLS], f32, tag="S24")

            # ---- load x: contiguous staging + PE transpose
            # Per 128-seq block (T, u): stg [128, 32], partition p = seq
            # offset within block, col = b*7 + c (cols 28..31 unused).
            # Transpose -> PSUM [32, 128]: partition = 7b + c, free = p.
            # Copy into X rows 32j + 7b + c (j = T//2) at the block's cols.
            for T in range(NTR):
                j = T // 2
                c0 = HALO + 512 * (T % 2)
                for u in range(4):
                    s0 = 512 * T + 128 * u
                    stg = pstg.tile([128, 32], f32, tag="stg")
                    eng = nc.sync if (T * 4 + u) % 2 == 0 else nc.scalar
                    eng.dma_start(
                        stg[:, 0:28],
                        x_d.ap()[:, s0 : s0 + 128, :].rearrange("b p c -> p b c"),
                    )
                    pst_t = psT.tile([32, 128], f32, tag="pst_t")
                    nc.tensor.transpose(pst_t[0:28, :], stg[:, 0:28], ident[:])
                    nc.scalar.copy(
                        X[32 * j : 32 * j + 28, c0 + 128 * u : c0 + 128 * (u + 1)],
                        pst_t[0:28, :],
                    )
                    if T % 2 == 1 and u == 3 and j + 1 < NJ:
                        # back-halo for chunk j+1: seq 1024(j+1)-23 .. -1
                        nc.scalar.copy(
                            X[32 * (j + 1) : 32 * (j + 1) + 28, 0:HALO],
                            pst_t[0:28, 128 - HALO : 128],
                        )
            # j=0 halo: replicate x[b,0,c] into cols 0..22
            nc.vector.tensor_scalar(
                X[0:28, 0:HALO],
                X[0:28, HALO : 2 * HALO],
                0.0,
                X[0:28, HALO : HALO + 1],
                Alu.mult,
                Alu.add,
            )

            # ---- per-stat contiguous ST2 [28, 4100] (GPSIMD copies)
            # ST2_t[7b+c, m] = feats_t[b, c, (m-2) mod 4096]
            ST2 = [
                pst2.tile([28, ST2W], f32, tag=f"ST2_{t}", name=f"ST2_{t}")
                for t in range(5)
            ]

            def relayout(t, st):
                st2 = ST2[t]
                for j in range(NJ):
                    nc.gpsimd.tensor_copy(
                        st2[:, 2 + CH * j : 2 + CH * (j + 1)],
                        st[32 * j : 32 * j + 28, HALO : HALO + CH],
                    )
                # wrap cols: 0:2 <- seq 4094..4095 ; 4098:4100 <- seq 0..1
                nc.gpsimd.tensor_copy(st2[:, 0:2], st[96:124, XCOLS - 2 : XCOLS])
                nc.gpsimd.tensor_copy(st2[:, S + 2 : S + 4], st[0:28, HALO : HALO + 2])

            relayout(0, X)  # raw x (ready right after the loader)

            # ---- rolling stats (all [128, *], shifts along free dim)
            E = XCOLS  # 1047

            def tt(dst, d0, a, a0, bsrc, b0, n, op):
                nc.vector.tensor_tensor(
                    dst[:, d0 : d0 + n], a[:, a0 : a0 + n], bsrc[:, b0 : b0 + n], op
                )

            # sum chain: A(T1) B(T2) C(T3) D(T1) S24
            tt(T1, 1, X, 1, X, 0, E - 1, Alu.add)
            tt(T2, 3, T1, 3, T1, 1, E - 3, Alu.add)
            tt(T3, 7, T2, 7, T2, 3, E - 7, Alu.add)
            tt(T1, 15, T3, 15, T3, 7, E - 15, Alu.add)
            tt(S24, 23, T1, 23, T3, 7, E - 23, Alu.add)
            relayout(1, S24)  # mean (raw window sum; /24 folded into weights)
            # squares: SQX(T4 via ACT), A2(T1) B2(T2) C2(T3) D2(T1) SQ24(T2)
            nc.scalar.square(T4[:, 0:E], X[:, 0:E])
            tt(T1, 1, T4, 1, T4, 0, E - 1, Alu.add)
            tt(T2, 3, T1, 3, T1, 1, E - 3, Alu.add)
            tt(T3, 7, T2, 7, T2, 3, E - 7, Alu.add)
            tt(T1, 15, T3, 15, T3, 7, E - 15, Alu.add)
            tt(T2, 23, T1, 23, T3, 7, E - 23, Alu.add)
            # mean^2*24 = (S24/sqrt(24))^2 -> T4 ; var_raw(T1) ; clamp(T2) ; SD
            nc.scalar.activation(
                T4[:, 23:E], S24[:, 23:E], Act.Square, 0.0, float(W**-0.5)
            )
            tt(T3, 23, T2, 23, T4, 23, E - 23, Alu.subtract)
            nc.vector.tensor_scalar(T2[:, 23:E], T3[:, 23:E], 0.0, None, Alu.max)
            nc.scalar.activation(T4[:, 23:E], T2[:, 23:E], Act.Sqrt, 0.0, 1.0 / (W - 1))
            relayout(4, T4)  # std
            # max chain: MA(T1) MB(T3) MC(T1) MD(T3) MX
            tt(T1, 1, X, 1, X, 0, E - 1, Alu.max)
            tt(T3, 3, T1, 3, T1, 1, E - 3, Alu.max)
            tt(T1, 7, T3, 7, T3, 3, E - 7, Alu.max)
            tt(T3, 15, T1, 15, T1, 7, E - 15, Alu.max)
            tt(T2, 23, T3, 23, T1, 7, E - 23, Alu.max)
            relayout(2, T2)  # max
            # min chain: NA(T4) NB(T2) NC(T4) ND(T2) MN
            tt(T1, 1, X, 1, X, 0, E - 1, Alu.min)
            tt(T3, 3, T1, 3, T1, 1, E - 3, Alu.min)
            tt(T1, 7, T3, 7, T3, 3, E - 7, Alu.min)
            tt(T3, 15, T1, 15, T1, 7, E - 15, Alu.min)
            tt(T4, 23, T3, 23, T1, 7, E - 23, Alu.min)
            relayout(3, T4)  # min

            # ---- per batch: build F3 (block k = ST2[:, k:k+4098]) + matmuls
            for b in range(BSH):
                f3 = pf3.tile([K, F3W], f32r, tag="F3")
                nc.sync.dma_start(f3[K - 1 : K, :], ones_d.ap())
                for t in range(5):
                    for k in range(3):
                        r0 = 35 * k + 7 * t
                        nc.sync.dma_start(
                            f3[r0 : r0 + 7, :],
                            ST2[t][7 * b : 7 * b + 7, k : k + F3W].bitcast(f32r),
                        )
                for g in range(NT // OUTG):
                    stage = pout.tile([128, OUTG * D], f32, tag="stage")
                    for q in range(OUTG):
                        t0 = 128 * (OUTG * g + q)
                        ps = pps.tile([128, D], f32, tag="ps")
                        nc.tensor.matmul(
                            ps[:],
                            f3[:, t0 + 1 : t0 + 129],
                            wt[:],
                            start=True,
                            stop=True,
                        )
                        c0 = D * q
                        nc.vector.tensor_copy(
                            stage[:, c0 : c0 + DVE_COLS], ps[:, 0:DVE_COLS]
                        )
                        nc.scalar.copy(
                            stage[:, c0 + DVE_COLS : c0 + D], ps[:, DVE_COLS:D]
                        )
                    nc.sync.dma_start(
                        y_d.ap()[
                            b, 128 * OUTG * g : 128 * OUTG * (g + 1), :
                        ].rearrange("(q p) d -> p q d", p=128),
                        stage[:].rearrange("p (q d) -> p q d", q=OUTG),
                    )

    nc.compile()
    return nc


def _prep_host(W_conv, b_conv):
    wt = np.empty((K, D), np.float32)
    wkf = np.ascontiguousarray(W_conv.transpose(2, 1, 0)).copy()  # (3, 35, 512)
    wkf[:, C : 2 * C, :] *= 1.0 / W  # fold mean = S24/24 into weights
    wt[: K - 1] = wkf.reshape(3 * NF, D)
    wt[K - 1] = b_conv.astype(np.float32)
    return wt


def _run(x, W_conv, b_conv, trace=False, **kw):
    from concourse import bass_utils

    if "nc" not in _CACHE:
        _CACHE["nc"] = _build()
    nc = _CACHE["nc"]

    wt = _prep_host(np.asarray(W_conv), np.asarray(b_conv))
    ones = np.ones((1, F3W), np.float32)
    ident = np.eye(128, dtype=np.float32)
    x = np.ascontiguousarray(np.asarray(x, np.float32))
    in_maps = [
        {"x": x[BSH * i : BSH * (i + 1)], "wt": wt, "ones": ones, "ident": ident}
        for i in range(NCORES)
    ]
    res = bass_utils.run_bass_kernel_spmd(
        nc, in_maps, core_ids=list(range(NCORES)), trace=trace, **kw
    )
    out = np.concatenate([r["y"] for r in res.results], axis=0)
    return out, res


def kernel(x, x_mark=None, W_conv=None, b_conv=None, **_unused):
    out, _ = _run(x, W_conv, b_conv, trace=False)
    return out


# revision 9
# speedup vs baseline: 2.3926x; 1.5639x over previous
"""Trainium2 Bass kernel for nn_DataEmbedding_Stats.

Computation: rolling-window stats (window=24, replicate-padded) over
x (B,S,7) -> 35 features -> circular conv1d(k=3) -> (B,S,512).

Strategy (8 NeuronCores, pure data parallel over batch, 4 batches/core):
 - x loaded contiguously (28B runs) into [128,128] staging tiles, then
   PE-transposed so channels land on partitions: X [128, 1047] layout,
   partition = 32j + 7b + c (j = 1024-seq chunk, b = local batch,
   c = channel), free = seq within chunk + 23-halo.
 - rolling sum/sumsq/max/min via log-doubling shifted ops on DVE
   (window 24 = combine(16-window, 8-window shifted by 16)).
 - std = sqrt(max(SQ24 - S24^2/24, 0)/23); mean folded into conv weights
   as S24 * (W_mean/24) on host.
 - per-stat contiguous ST2 [28, 4100] tiles (partition = 7b+c, col m =
   feats at seq (m-2) mod 4096) built with GPSIMD copies; F3 [106, 4098]
   per batch = 3 conv-tap shifts of the 35 features (circular) + ones
   row (bias folded as contraction row 105) via 15 wide DMAs per batch.
 - conv as matmul: per 128 positions, out[128,512] = F3[:,t+1:t+129].T
   @ Wt, float32r operands -> full PE speed at N=512.
 - PSUM -> SBUF copies split between DVE/ACT, 2MB output DMAs.
"""

import numpy as np

try:
    import concourse.bass as bass  # noqa: F401
except ImportError:
    import sys

    for _p in ("/opt/trn_rl_repo", "/root/.axon_site/_ro/trn_rl_repo"):
        if _p not in sys.path:
            sys.path.insert(0, _p)

B, S, C, W, D = 32, 4096, 7, 24, 512
NCORES = 8
BSH = B // NCORES          # batches per core
NJ = 4                     # seq chunks (row groups of 32 partitions)
CH = S // NJ               # 1024
HALO = W - 1               # 23
XCOLS = CH + HALO          # 1047
NF = 5 * C                 # 35 features
K = 3 * NF + 1             # 106 contraction rows (ones row last)
F3W = S + 2                # 4098
ST2W = S + 4               # 4100: col m = feats[(m-2) mod S]
NT = S // 128              # 32 position tiles per batch
NTR = S // 512             # 8 PE transposes (each covers 512 seq x 4 batch)
DVE_COLS = 224             # psum-copy split: DVE [0:224], ACT [224:512]
OUTG = 8                   # position tiles per output staging tile

_CACHE = {}


def _build():
    import concourse.bacc as bacc
    import concourse.tile as tile
    from concourse import mybir

    f32 = mybir.dt.float32
    bf16 = mybir.dt.bfloat16
    Alu = mybir.AluOpType
    Act = mybir.ActivationFunctionType

    nc = bacc.Bacc(
        "TRN2",
        target_bir_lowering=False,
        debug=False,
        enable_asserts=False,
        num_devices=NCORES,
    )

    x_d = nc.dram_tensor("x", (BSH, S, C), f32, kind="ExternalInput")
    wt_d = nc.dram_tensor("wt", (K, D), bf16, kind="ExternalInput")
    ones_d = nc.dram_tensor("ones", (1, F3W), bf16, kind="ExternalInput")
    id_d = nc.dram_tensor("ident", (128, 128), f32, kind="ExternalInput")
    y_d = nc.dram_tensor("y", (BSH, S, D), f32, kind="ExternalOutput")

    with tile.TileContext(nc) as tc:
        with (
            tc.tile_pool(name="stats", bufs=1) as pst,
            tc.tile_pool(name="st2p", bufs=1) as pst2,
            tc.tile_pool(name="f3p", bufs=2) as pf3,
            tc.tile_pool(name="wtp", bufs=1) as pwt,
            tc.tile_pool(name="stage_in", bufs=3) as pstg,
            tc.tile_pool(name="psT", bufs=2, space="PSUM") as psT,
            tc.tile_pool(name="psum", bufs=6, space="PSUM") as pps,
            tc.tile_pool(name="outp", bufs=2) as pout,
        ):
            wt = pwt.tile([K, D], bf16, tag="wt")
            nc.sync.dma_start(wt[:], wt_d.ap())
            ident = pwt.tile([128, 128], f32, tag="ident")
            nc.sync.dma_start(ident[:], id_d.ap())

            X = pst.tile([128, XCOLS], f32, tag="X")
            T1 = pst.tile([128, XCOLS], f32, tag="T1")
            T2 = pst.tile([128, XCOLS], f32, tag="T2")
            T3 = pst.tile([128, XCOLS], f32, tag="T3")
            T4 = pst.tile([128, XCOLS], f32, tag="T4")
            S24 = pst.tile([128, XCOLS], f32, tag="S24")

            # ---- load x: contiguous staging + PE transpose
            # Per 128-seq block (T, u): stg [128, 32], partition p = seq
            # offset within block, col = b*7 + c (cols 28..31 unused).
            # Transpose -> PSUM [32, 128]: partition = 7b + c, free = p.
            # Copy into X rows 32j + 7b + c (j = T//2) at the block's cols.
            for T in range(NTR):
                j = T // 2
                c0 = HALO + 512 * (T % 2)
                for u in range(4):
                    s0 = 512 * T + 128 * u
                    stg = pstg.tile([128, 32], f32, tag="stg")
                    eng = nc.sync if (T * 4 + u) % 2 == 0 else nc.scalar
                    eng.dma_start(
                        stg[:, 0:28],
                        x_d.ap()[:, s0 : s0 + 128, :].rearrange("b p c -> p b c"),
                    )
                    pst_t = psT.tile([32, 128], f32, tag="pst_t")
                    nc.tensor.transpose(pst_t[0:28, :], stg[:, 0:28], ident[:])
                    nc.scalar.copy(
                        X[32 * j : 32 * j + 28, c0 + 128 * u : c0 + 128 * (u + 1)],
                        pst_t[0:28, :],
                    )
                    if T % 2 == 1 and u == 3 and j + 1 < NJ:
                        # back-halo for chunk j+1: seq 1024(j+1)-23 .. -1
                        nc.scalar.copy(
                            X[32 * (j + 1) : 32 * (j + 1) + 28, 0:HALO],
                            pst_t[0:28, 128 - HALO : 128],
                        )
            # j=0 halo: replicate x[b,0,c] into cols 0..22
            nc.vector.tensor_scalar(
                X[0:28, 0:HALO],
                X[0:28, HALO : 2 * HALO],
                0.0,
                X[0:28, HALO : HALO + 1],
                Alu.mult,
                Alu.add,
            )

            # ---- per-stat contiguous ST2 [28, 4100] (GPSIMD copies)
            # ST2_t[7b+c, m] = feats_t[b, c, (m-2) mod 4096]
            ST2 = [
                pst2.tile([28, ST2W], bf16, tag=f"ST2_{t}", name=f"ST2_{t}")
                for t in range(5)
            ]

            def relayout(t, st):
                st2 = ST2[t]
                for j in range(NJ):
                    nc.scalar.copy(
                        st2[:, 2 + CH * j : 2 + CH * (j + 1)],
                        st[32 * j : 32 * j + 28, HALO : HALO + CH],
                    )
                # wrap cols: 0:2 <- seq 4094..4095 ; 4098:4100 <- seq 0..1
                nc.scalar.copy(st2[:, 0:2], st[96:124, XCOLS - 2 : XCOLS])
                nc.scalar.copy(st2[:, S + 2 : S + 4], st[0:28, HALO : HALO + 2])

            relayout(0, X)  # raw x (ready right after the loader)

            # ---- rolling stats (all [128, *], shifts along free dim)
            E = XCOLS  # 1047

            def tt(dst, d0, a, a0, bsrc, b0, n, op):
                nc.vector.tensor_tensor(
                    dst[:, d0 : d0 + n], a[:, a0 : a0 + n], bsrc[:, b0 : b0 + n], op
                )

            # sum chain: A(T1) B(T2) C(T3) D(T1) S24
            tt(T1, 1, X, 1, X, 0, E - 1, Alu.add)
            tt(T2, 3, T1, 3, T1, 1, E - 3, Alu.add)
            tt(T3, 7, T2, 7, T2, 3, E - 7, Alu.add)
            tt(T1, 15, T3, 15, T3, 7, E - 15, Alu.add)
            tt(S24, 23, T1, 23, T3, 7, E - 23, Alu.add)
            relayout(1, S24)  # mean (raw window sum; /24 folded into weights)
            # squares: SQX(T4 via ACT), A2(T1) B2(T2) C2(T3) D2(T1) SQ24(T2)
            nc.scalar.square(T4[:, 0:E], X[:, 0:E])
            tt(T1, 1, T4, 1, T4, 0, E - 1, Alu.add)
            tt(T2, 3, T1, 3, T1, 1, E - 3, Alu.add)
            tt(T3, 7, T2, 7, T2, 3, E - 7, Alu.add)
            tt(T1, 15, T3, 15, T3, 7, E - 15, Alu.add)
            tt(T2, 23, T1, 23, T3, 7, E - 23, Alu.add)
            # mean^2*24 = (S24/sqrt(24))^2 -> T4 ; var_raw(T1) ; clamp(T2) ; SD
            nc.scalar.activation(
                T4[:, 23:E], S24[:, 23:E], Act.Square, 0.0, float(W**-0.5)
            )
            tt(T3, 23, T2, 23, T4, 23, E - 23, Alu.subtract)
            nc.vector.tensor_scalar(T2[:, 23:E], T3[:, 23:E], 0.0, None, Alu.max)
            nc.scalar.activation(T4[:, 23:E], T2[:, 23:E], Act.Sqrt, 0.0, 1.0 / (W - 1))
            relayout(4, T4)  # std
            # max chain: MA(T1) MB(T3) MC(T1) MD(T3) MX
            tt(T1, 1, X, 1, X, 0, E - 1, Alu.max)
            tt(T3, 3, T1, 3, T1, 1, E - 3, Alu.max)
            tt(T1, 7, T3, 7, T3, 3, E - 7, Alu.max)
            tt(T3, 15, T1, 15, T1, 7, E - 15, Alu.max)
            tt(T2, 23, T3, 23, T1, 7, E - 23, Alu.max)
            relayout(2, T2)  # max
            # min chain: NA(T4) NB(T2) NC(T4) ND(T2) MN
            tt(T1, 1, X, 1, X, 0, E - 1, Alu.min)
            tt(T3, 3, T1, 3, T1, 1, E - 3, Alu.min)
            tt(T1, 7, T3, 7, T3, 3, E - 7, Alu.min)
            tt(T3, 15, T1, 15, T1, 7, E - 15, Alu.min)
            tt(T4, 23, T3, 23, T1, 7, E - 23, Alu.min)
            relayout(3, T4)  # min

            # ---- per batch: build F3 (block k = ST2[:, k:k+4098]) + matmuls
            for b in range(BSH):
                f3 = pf3.tile([K, F3W], bf16, tag="F3")
                nc.sync.dma_start(f3[K - 1 : K, :], ones_d.ap())
                for t in range(5):
                    for k in range(3):
                        r0 = 35 * k + 7 * t
                        eng = nc.sync if (t * 3 + k) % 2 == 0 else nc.scalar
                        eng.dma_start(
                            f3[r0 : r0 + 7, :],
                            ST2[t][7 * b : 7 * b + 7, k : k + F3W],
                        )
                for g in range(NT // OUTG):
                    stage = pout.tile([128, OUTG * D], f32, tag="stage")
                    for q in range(OUTG):
                        t0 = 128 * (OUTG * g + q)
                        ps = pps.tile([128, D], f32, tag="ps")
                        nc.tensor.matmul(
                            ps[:],
                            f3[:, t0 + 1 : t0 + 129],
                            wt[:],
                            start=True,
                            stop=True,
                        )
                        c0 = D * q
                        nc.vector.tensor_copy(
                            stage[:, c0 : c0 + DVE_COLS], ps[:, 0:DVE_COLS]
                        )
                        nc.scalar.copy(
                            stage[:, c0 + DVE_COLS : c0 + D], ps[:, DVE_COLS:D]
                        )
                    nc.sync.dma_start(
                        y_d.ap()[
                            b, 128 * OUTG * g : 128 * OUTG * (g + 1), :
                        ].rearrange("(q p) d -> p q d", p=128),
                        stage[:].rearrange("p (q d) -> p q d", q=OUTG),
                    )

    nc.compile()
    return nc


def _prep_host(W_conv, b_conv):
    import ml_dtypes

    wt = np.empty((K, D), np.float32)
    wkf = np.ascontiguousarray(W_conv.transpose(2, 1, 0)).copy()  # (3, 35, 512)
    wkf[:, C : 2 * C, :] *= 1.0 / W  # fold mean = S24/24 into weights
    wt[: K - 1] = wkf.reshape(3 * NF, D)
    wt[K - 1] = b_conv.astype(np.float32)
    return wt.astype(ml_dtypes.bfloat16)


def _run(x, W_conv, b_conv, trace=False, **kw):
    from concourse import bass_utils

    if "nc" not in _CACHE:
        _CACHE["nc"] = _build()
    nc = _CACHE["nc"]

    wt = _prep_host(np.asarray(W_conv), np.asarray(b_conv))
    import ml_dtypes

    ones = np.ones((1, F3W), ml_dtypes.bfloat16)
    ident = np.eye(128, dtype=np.float32)
    x = np.ascontiguousarray(np.asarray(x, np.float32))
    in_maps = [
        {"x": x[BSH * i : BSH * (i + 1)], "wt": wt, "ones": ones, "ident": ident}
        for i in range(NCORES)
    ]
    res = bass_utils.run_bass_kernel_spmd(
        nc, in_maps, core_ids=list(range(NCORES)), trace=trace, **kw
    )
    out = np.concatenate([r["y"] for r in res.results], axis=0)
    return out, res


def kernel(x, x_mark=None, W_conv=None, b_conv=None, **_unused):
    out, _ = _run(x, W_conv, b_conv, trace=False)
    return out
